# revision 15
# baseline (speedup 1.0000x reference)
"""Trainium2 Bass kernel for a DeepSeek-style MoE block (expert-parallel over 8 cores).

Strategy:
  - Each core owns one expert (8 experts / 8 cores). x + router weights are
    replicated; c_fc/c_proj are sharded along the expert axis.
  - Every core computes the full router (logits -> top-2 -> capacity ranking)
    on-device (cheap), builds its own expert's slot->(token, weight) table via
    an indirect-DMA scatter, gathers its tokens, runs the expert MLP, and
    scatters weighted outputs into a [N, D] partial buffer.
  - A ReduceScatter across the 8 cores combines partials; each core LayerNorms
    its 1/8 shard of tokens and returns it.  The host concatenates shards.

Matmul orientation: activations are kept feature-major (transposed) so both
expert weights act as natively-laid-out operands:
  hT[f, c]  = sum_d c_fc[d, f] * xbT[d, c]      (lhsT = c_fc slab, rhs = xbT)
  eo[c, d]  = sum_f hT[f, c]  * c_proj[f, d]    (lhsT = hT slice,  rhs = c_proj slab)
"""

import os
import sys
from contextlib import ExitStack

import numpy as np

for _p in ("/opt/trn_rl_repo", "/root/.axon_site/_ro/trn_rl_repo"):
    if os.path.isdir(_p) and _p not in sys.path:
        sys.path.insert(0, _p)

P = 128

FULL_CFG = dict(N=4096, D=1024, E=8, CAP=2048, CBLK=1024, n_cores=8,
                act="Gelu", ln_eps=1e-5)


def build_moe_kernel(N, D, E, CAP, CBLK, n_cores, act="Gelu", ln_eps=1e-5,
                     debug_taps=False):
    """Builds and compiles the SPMD Bass kernel. Returns the Bacc object."""
    from concourse import bacc, bass, mybir
    import concourse.tile as tile
    from concourse.masks import make_identity, make_upper_triangular

    FP32 = mybir.dt.float32
    BF16 = mybir.dt.bfloat16
    I32 = mybir.dt.int32
    AF = mybir.ActivationFunctionType
    ALU = mybir.AluOpType
    X = mybir.AxisListType.X
    IOA = bass.IndirectOffsetOnAxis

    F = 4 * D
    NCH = N // P           # token chunks
    KD = D // P            # contraction chunks for mm1
    FCH = F // P           # f chunks
    B2 = 2 * NCH           # (slot k, token-chunk) blocks in rank order
    NBLK = CAP // CBLK     # capacity blocks
    MCH = CBLK // P        # slot chunks per capacity block
    CAPCH = CAP // P       # slot chunks total
    DHW = min(512, D)      # mm2 output width per matmul
    NDH = D // DHW
    HHW = min(512, CBLK)   # mm1 output width per matmul
    NHH = CBLK // HHW
    NSH = N // n_cores     # output shard rows per core
    NB512 = N // 512       # router column blocks
    act_fn = getattr(AF, act)
    assert N % 512 == 0 and B2 * E <= 512 and D % 512 in (0, D)

    nc = bacc.Bacc("TRN2", target_bir_lowering=False, debug=False,
                   num_devices=n_cores)

    xpad = nc.dram_tensor("xpad", [N + 1, D], FP32, kind="ExternalInput").ap()
    xT = nc.dram_tensor("xT", [D, N], FP32, kind="ExternalInput").ap()
    wg = nc.dram_tensor("wg", [D, E], FP32, kind="ExternalInput").ap()
    cfc = nc.dram_tensor("cfc", [FCH, P, KD, P], FP32, kind="ExternalInput").ap()
    cpj = nc.dram_tensor("cpj", [NDH, FCH, P, DHW], BF16, kind="ExternalInput").ap()
    ebias = nc.dram_tensor("ebias", [P, B2 * E], FP32, kind="ExternalInput").ap()
    lnw = nc.dram_tensor("lnw", [P, D], FP32, kind="ExternalInput").ap()
    lnb = nc.dram_tensor("lnb", [P, D], FP32, kind="ExternalInput").ap()
    out_ext = nc.dram_tensor("out", [NSH, D], FP32, kind="ExternalOutput").ap()
    if debug_taps:
        dbg_logits = nc.dram_tensor("dbg_logits", [P, NCH * E], FP32,
                                    kind="ExternalOutput").ap()
        dbg_rnk = nc.dram_tensor("dbg_rnk", [P, B2 * E], FP32,
                                 kind="ExternalOutput").ap()
        dbg_dest = nc.dram_tensor("dbg_dest", [P, B2], FP32,
                                  kind="ExternalOutput").ap()
        dbg_tokw = nc.dram_tensor("dbg_tokw", [P, CAPCH * 2], FP32,
                                  kind="ExternalOutput").ap()
        dbg_xbt = nc.dram_tensor("dbg_xbt", [P, KD * CBLK], FP32,
                                 kind="ExternalOutput").ap()
        dbg_ht = nc.dram_tensor("dbg_ht", [P, FCH * CBLK], FP32,
                                kind="ExternalOutput").ap()
        dbg_partial = nc.dram_tensor("dbg_partial", [N + 1, D], FP32,
                                     kind="ExternalOutput").ap()
        dbg_rs = nc.dram_tensor("dbg_rs", [NSH, D], FP32,
                                kind="ExternalOutput").ap()

    with tile.TileContext(nc) as tc:
        with ExitStack() as root:
            dram = root.enter_context(tc.tile_pool(name="dram", bufs=1, space="DRAM"))
            ps = root.enter_context(tc.tile_pool(name="ps", bufs=8, space="PSUM"))
            const = root.enter_context(tc.tile_pool(name="const", bufs=1))

            # slot -> (token, weight) table; row CAP is the trash row
            table = dram.tile([CAP + 1, 2], FP32)
            # per-core partial output; row N is the trash row
            partial = dram.tile([N + 1, D], FP32)
            rs_out = dram.tile([NSH, D], FP32)

            ident = const.tile([P, P], FP32)
            make_identity(nc, ident[:])
            ustrict = const.tile([P, P], FP32)   # U[k, m] = 1 iff m > k
            make_upper_triangular(nc, ustrict[:], val=1.0, diag=False)
            ones_t = const.tile([P, P], FP32)
            nc.vector.memset(ones_t[:], 1.0)

            # ---------------- router ----------------
            with ExitStack() as rt_scope:
                rt = rt_scope.enter_context(tc.tile_pool(name="rt", bufs=1))

                wg_sb = rt.tile([P, KD, E], FP32)
                nc.sync.dma_start(out=wg_sb[:], in_=wg.rearrange("(k p) e -> p k e", p=P))
                eb_sb = rt.tile([P, B2 * E], FP32)
                nc.sync.dma_start(out=eb_sb[:], in_=ebias[:])

                # logits[n, e] computed as (w_g^T @ x^T)^T in 512-token blocks
                logits = rt.tile([P, NCH, E], FP32)
                for nb in range(NB512):
                    ps_lt = ps.tile([P, 512], FP32, tag="ps")
                    for k in range(KD):
                        xt_sb = rt.tile([P, 512], FP32, tag="xt", bufs=2)
                        nc.sync.dma_start(out=xt_sb[:], in_=xT[k * P:(k + 1) * P,
                                                             nb * 512:(nb + 1) * 512])
                        nc.tensor.matmul(out=ps_lt[:E, :], lhsT=wg_sb[:, k, :],
                                         rhs=xt_sb[:], start=(k == 0), stop=(k == KD - 1))
                    lt_sb = rt.tile([E, 512], FP32, tag="lt", bufs=2)
                    nc.vector.tensor_copy(out=lt_sb[:], in_=ps_lt[:E, :])
                    for i in range(4):  # 512 tokens -> 4 chunks of 128
                        ps_t = ps.tile([P, 512], FP32, tag="ps")
                        nc.tensor.transpose(out=ps_t[:, :E], in_=lt_sb[:, i * P:(i + 1) * P],
                                            identity=ident[:E, :E])
                        nc.vector.tensor_copy(out=logits[:, nb * 4 + i, :], in_=ps_t[:, :E])

                if debug_taps:
                    nc.sync.dma_start(out=dbg_logits[:],
                                      in_=logits[:].rearrange("p a e -> p (a e)"))
                # top-2 over experts
                v0 = rt.tile([P, NCH], FP32)
                nc.vector.tensor_reduce(out=v0[:], in_=logits[:], axis=X, op=ALU.max)
                mask01 = rt.tile([P, B2, E], FP32)
                nc.vector.tensor_tensor(out=mask01[:, :NCH, :], in0=logits[:],
                                        in1=v0[:].unsqueeze(2).to_broadcast([P, NCH, E]),
                                        op=ALU.is_equal)
                mbig = rt.tile([P, NCH, E], FP32)
                nc.vector.tensor_scalar(out=mbig[:], in0=mask01[:, :NCH, :],
                                        scalar1=1e30, scalar2=None, op0=ALU.mult)
                lm = rt.tile([P, NCH, E], FP32)
                nc.vector.tensor_tensor(out=lm[:], in0=logits[:], in1=mbig[:], op=ALU.subtract)
                v1 = rt.tile([P, NCH], FP32)
                nc.vector.tensor_reduce(out=v1[:], in_=lm[:], axis=X, op=ALU.max)
                nc.vector.tensor_tensor(out=mask01[:, NCH:, :], in0=lm[:],
                                        in1=v1[:].unsqueeze(2).to_broadcast([P, NCH, E]),
                                        op=ALU.is_equal)

                # softmax over the two selected logits
                dv = rt.tile([P, NCH], FP32)
                nc.vector.tensor_tensor(out=dv[:], in0=v1[:], in1=v0[:], op=ALU.subtract)
                p1 = rt.tile([P, NCH], FP32)
                nc.scalar.activation(out=p1[:], in_=dv[:], func=AF.Exp)
                z = rt.tile([P, NCH], FP32)
                nc.vector.tensor_scalar(out=z[:], in0=p1[:], scalar1=1.0, scalar2=None,
                                        op0=ALU.add)
                vw = rt.tile([P, B2], FP32)
                w0v = rt.tile([P, NCH], FP32)
                nc.vector.reciprocal(out=w0v[:], in_=z[:])
                nc.vector.tensor_copy(out=vw[:, :NCH], in_=w0v[:])
                nc.vector.tensor_tensor(out=vw[:, NCH:], in0=p1[:], in1=w0v[:], op=ALU.mult)

                # exclusive cumsum over flattened (k, n) per expert:
                # intra-chunk via strictly-upper-triangular matmul, chunk offsets
                # via a log-step scan over per-chunk column sums
                ps_s = ps.tile([P, 512], FP32, tag="ps")
                nc.tensor.matmul(out=ps_s[:, :B2 * E], lhsT=ustrict[:], rhs=mask01[:],
                                 start=True, stop=True)
                ps_c = ps.tile([P, 512], FP32, tag="ps")
                nc.tensor.matmul(out=ps_c[:, :B2 * E], lhsT=ones_t[:], rhs=mask01[:],
                                 start=True, stop=True)
                ea = rt.tile([P, B2 * E], FP32)
                eb2 = rt.tile([P, B2 * E], FP32)
                nc.vector.memset(ea[:, :E], 0.0)
                nc.vector.tensor_copy(out=ea[:, E:], in_=ps_c[:, :(B2 - 1) * E])
                cur, nxt = ea, eb2
                s = 1
                while s < B2:
                    w = s * E
                    nc.vector.tensor_copy(out=nxt[:, :w], in_=cur[:, :w])
                    nc.vector.tensor_tensor(out=nxt[:, w:B2 * E], in0=cur[:, w:B2 * E],
                                            in1=cur[:, :B2 * E - w], op=ALU.add)
                    cur, nxt = nxt, cur
                    s *= 2
                rnk = rt.tile([P, B2 * E], FP32)
                nc.vector.tensor_tensor(out=rnk[:], in0=ps_s[:, :B2 * E], in1=cur[:],
                                        op=ALU.add)

                if debug_taps:
                    nc.sync.dma_start(out=dbg_rnk[:], in_=rnk[:])
                # keep mask (capacity drop) and scatter destination
                klt = rt.tile([P, B2 * E], FP32)
                nc.vector.tensor_scalar(out=klt[:], in0=rnk[:], scalar1=float(CAP),
                                        scalar2=None, op0=ALU.is_lt)
                kept = rt.tile([P, B2 * E], FP32)
                nc.vector.tensor_tensor(out=kept[:], in0=klt[:],
                                        in1=mask01[:].rearrange("p b e -> p (b e)"),
                                        op=ALU.mult)
                av = rt.tile([P, B2 * E], FP32)
                nc.vector.tensor_tensor(out=av[:], in0=rnk[:], in1=eb_sb[:], op=ALU.add)
                pm = rt.tile([P, B2 * E], FP32)
                nc.vector.tensor_tensor(out=pm[:], in0=kept[:], in1=av[:], op=ALU.mult)
                draw = rt.tile([P, B2], FP32)
                nc.vector.tensor_reduce(out=draw[:], in_=pm[:].rearrange("p (b e) -> p b e", e=E),
                                        axis=X, op=ALU.add)
                s1 = rt.tile([P, B2], FP32)
                nc.vector.tensor_reduce(out=s1[:], in_=kept[:].rearrange("p (b e) -> p b e", e=E),
                                        axis=X, op=ALU.add)
                t2 = rt.tile([P, B2], FP32)
                nc.vector.tensor_scalar(out=t2[:], in0=s1[:], scalar1=-float(CAP),
                                        scalar2=float(CAP), op0=ALU.mult, op1=ALU.add)
                destf = rt.tile([P, B2], FP32)
                nc.vector.tensor_tensor(out=destf[:], in0=draw[:], in1=t2[:], op=ALU.add)
                destc = rt.tile([P, B2], FP32)
                nc.vector.tensor_scalar(out=destc[:], in0=destf[:], scalar1=float(CAP),
                                        scalar2=0.0, op0=ALU.min, op1=ALU.max)
                didx = rt.tile([P, B2], I32)
                nc.vector.tensor_copy(out=didx[:], in_=destc[:])
                if debug_taps:
                    nc.sync.dma_start(out=dbg_dest[:], in_=destc[:])

                # (token, weight) pair staging + table init + scatter
                vtok = rt.tile([P, B2], I32)
                nc.gpsimd.iota(vtok[:], pattern=[[0, 2], [P, NCH]], base=0,
                               channel_multiplier=1)
                stage = rt.tile([P, B2, 2], FP32)
                nc.vector.tensor_copy(out=stage[:, :, 0:1], in_=vtok[:].unsqueeze(2))
                nc.vector.tensor_copy(out=stage[:, :, 1:2], in_=vw[:].unsqueeze(2))
                tinit = rt.tile([P, CAPCH, 2], FP32)
                nc.vector.memset(tinit[:, :, 0:1], float(N))
                nc.vector.memset(tinit[:, :, 1:2], 0.0)
                nc.sync.dma_start(out=table[0:CAP, :].rearrange("(i p) c -> p i c", p=P),
                                  in_=tinit[:])
                # one scatter per (slot, token-chunk) column: the HW SWDGE
                # indirect ucode only honors one index per partition
                for b in range(B2):
                    nc.gpsimd.indirect_dma_start(
                        out=table[:], out_offset=IOA(ap=didx[:, b:b + 1], axis=0),
                        in_=stage[:, b, :], in_offset=None)

                # zero the partial-output buffer
                zf = rt.tile([P, D], FP32)
                nc.vector.memset(zf[:], 0.0)
                for i in range(NCH):
                    nc.sync.dma_start(out=partial[i * P:(i + 1) * P, :], in_=zf[:])

            # ---------------- slot table read-back ----------------
            slot = root.enter_context(tc.tile_pool(name="slot", bufs=1))
            tokw = slot.tile([P, CAPCH, 2], FP32)
            nc.sync.dma_start(out=tokw[:],
                              in_=table[0:CAP, :].rearrange("(i p) c -> p i c", p=P))
            tok_i = slot.tile([P, CAPCH], I32)
            nc.vector.tensor_copy(out=tok_i[:].unsqueeze(2), in_=tokw[:, :, 0:1])
            wsl = slot.tile([P, CAPCH], FP32)
            nc.vector.tensor_copy(out=wsl[:].unsqueeze(2), in_=tokw[:, :, 1:2])
            if debug_taps:
                nc.sync.dma_start(out=dbg_tokw[:],
                                  in_=tokw[:].rearrange("p i c -> p (i c)"))

            # ---------------- expert compute ----------------
            with ExitStack() as mn_scope:
                mn = mn_scope.enter_context(tc.tile_pool(name="mn", bufs=1))
                xbT = mn.tile([P, KD, CBLK], FP32)
                hT = mn.tile([P, FCH, CBLK], BF16)
                for blk in range(NBLK):
                    # gather this block's tokens, then transpose to feature-major
                    for m in range(MCH):
                        xb = mn.tile([P, D], FP32, tag="xb", bufs=2)
                        c0 = blk * MCH + m
                        nc.gpsimd.indirect_dma_start(
                            out=xb[:], out_offset=None, in_=xpad[:],
                            in_offset=IOA(ap=tok_i[:, c0:c0 + 1], axis=0))
                        for kd in range(KD):
                            ps_t = ps.tile([P, 512], FP32, tag="ps")
                            nc.tensor.transpose(out=ps_t[:, :P],
                                                in_=xb[:, kd * P:(kd + 1) * P],
                                                identity=ident[:])
                            nc.vector.tensor_copy(out=xbT[:, kd, m * P:(m + 1) * P],
                                                  in_=ps_t[:, :P])
                    if debug_taps and blk == 0:
                        nc.sync.dma_start(out=dbg_xbt[:],
                                          in_=xbT[:].rearrange("p k c -> p (k c)"))
                    # mm1: hT = act(c_fc^T-contracted with xbT), f-major
                    for f in range(FCH):
                        cfc_sb = mn.tile([P, KD, P], FP32, tag="cfc", bufs=3)
                        nc.sync.dma_start(out=cfc_sb[:], in_=cfc[f])
                        hps = [ps.tile([P, 512], FP32, tag="ps", name=f"hps{hh}")
                               for hh in range(NHH)]
                        for kd in range(KD):
                            for hh in range(NHH):
                                nc.tensor.matmul(out=hps[hh][:, :HHW], lhsT=cfc_sb[:, kd, :],
                                                 rhs=xbT[:, kd, hh * HHW:(hh + 1) * HHW],
                                                 start=(kd == 0), stop=(kd == KD - 1))
                        for hh in range(NHH):
                            nc.scalar.activation(out=hT[:, f, hh * HHW:(hh + 1) * HHW],
                                                 in_=hps[hh][:, :HHW], func=act_fn)
                    if debug_taps and blk == 0:
                        nc.gpsimd.dma_start(out=dbg_ht[:],
                                            in_=hT[:].rearrange("p f c -> p (f c)"))
                    # mm2: eo[c, d] accumulated over f, then weighted + scattered
                    for dh in range(NDH):
                        cps = []
                        for f in range(FCH):
                            cp = mn.tile([P, DHW], BF16, tag="cpj", bufs=FCH + 2,
                                         name=f"cpj{f}")
                            nc.sync.dma_start(out=cp[:], in_=cpj[dh, f])
                            cps.append(cp)
                        for m in range(MCH):
                            gs = blk * MCH + m
                            pe_ps = ps.tile([P, 512], FP32, tag="ps")
                            for f in range(FCH):
                                nc.tensor.matmul(out=pe_ps[:, :DHW],
                                                 lhsT=hT[:, f, m * P:(m + 1) * P],
                                                 rhs=cps[f][:],
                                                 start=(f == 0), stop=(f == FCH - 1))
                            eo = mn.tile([P, DHW], FP32, tag="eo", bufs=4)
                            nc.vector.tensor_tensor(
                                out=eo[:], in0=pe_ps[:, :DHW],
                                in1=wsl[:, gs:gs + 1].to_broadcast([P, DHW]),
                                op=ALU.mult)
                            nc.gpsimd.indirect_dma_start(
                                out=partial[:], out_offset=IOA(ap=tok_i[:, gs:gs + 1], axis=0),
                                in_=eo[:], in_offset=None, element_offset=dh * DHW)

            # ---------------- combine + layernorm ----------------
            if debug_taps:
                nc.sync.dma_start(out=dbg_partial[:], in_=partial[:])
            nc.gpsimd.collective_compute(
                "ReduceScatter", mybir.AluOpType.add,
                replica_groups=[list(range(n_cores))],
                ins=[partial[0:N, :].opt()], outs=[rs_out.opt()])

            if debug_taps:
                nc.sync.dma_start(out=dbg_rs[:], in_=rs_out[:])
            with ExitStack() as ln_scope:
                lnp = ln_scope.enter_context(tc.tile_pool(name="ln", bufs=1))
                lnw_sb = lnp.tile([P, D], FP32)
                nc.sync.dma_start(out=lnw_sb[:], in_=lnw[:])
                lnb_sb = lnp.tile([P, D], FP32)
                nc.sync.dma_start(out=lnb_sb[:], in_=lnb[:])
                epsb = lnp.tile([P, 1], FP32)
                nc.vector.memset(epsb[:], float(ln_eps))
                nt = (NSH + P - 1) // P
                for i in range(nt):
                    rows = min(P, NSH - i * P)
                    xr = lnp.tile([P, D], FP32, tag="xr", bufs=2)
                    nc.sync.dma_start(out=xr[:rows, :], in_=rs_out[i * P:i * P + rows, :])
                    sm = lnp.tile([P, 1], FP32, tag="sm", bufs=2)
                    nc.vector.tensor_reduce(out=sm[:rows], in_=xr[:rows, :], axis=X, op=ALU.add)
                    mu = lnp.tile([P, 1], FP32, tag="mu", bufs=2)
                    nc.vector.tensor_scalar(out=mu[:rows], in0=sm[:rows], scalar1=1.0 / D,
                                            scalar2=None, op0=ALU.mult)
                    xc = lnp.tile([P, D], FP32, tag="xc", bufs=2)
                    nc.vector.tensor_scalar(out=xc[:rows], in0=xr[:rows, :], scalar1=mu[:rows],
                                            scalar2=None, op0=ALU.subtract)
                    sq = lnp.tile([P, D], FP32, tag="sq", bufs=2)
                    vs = lnp.tile([P, 1], FP32, tag="vs", bufs=2)
                    nc.scalar.activation(out=sq[:rows], in_=xc[:rows], func=AF.Square,
                                         accum_out=vs[:rows])
                    vr = lnp.tile([P, 1], FP32, tag="vr", bufs=2)
                    nc.vector.tensor_scalar(out=vr[:rows], in0=vs[:rows], scalar1=1.0 / D,
                                            scalar2=None, op0=ALU.mult)
                    sd = lnp.tile([P, 1], FP32, tag="sd", bufs=2)
                    nc.scalar.activation(out=sd[:rows], in_=vr[:rows], func=AF.Sqrt,
                                         bias=epsb[:rows])
                    rsd = lnp.tile([P, 1], FP32, tag="rsd", bufs=2)
                    nc.vector.reciprocal(out=rsd[:rows], in_=sd[:rows])
                    xn = lnp.tile([P, D], FP32, tag="xn", bufs=2)
                    nc.vector.tensor_scalar(out=xn[:rows], in0=xc[:rows], scalar1=rsd[:rows],
                                            scalar2=None, op0=ALU.mult)
                    y1 = lnp.tile([P, D], FP32, tag="y1", bufs=2)
                    nc.vector.tensor_tensor(out=y1[:rows], in0=xn[:rows], in1=lnw_sb[:rows, :],
                                            op=ALU.mult)
                    yo = lnp.tile([P, D], FP32, tag="yo", bufs=2)
                    nc.vector.tensor_tensor(out=yo[:rows], in0=y1[:rows], in1=lnb_sb[:rows, :],
                                            op=ALU.add)
                    nc.sync.dma_start(out=out_ext[i * P:i * P + rows, :], in_=yo[:rows, :])

    nc.compile()
    return nc


def prep_in_maps(x, w_g, c_fc, c_proj, ln_w, ln_b, cfg):
    """Host-side input prep: replication, padding, layout tiling, bf16 cast."""
    from concourse import mybir

    N, D, E, CAP, CBLK = cfg["N"], cfg["D"], cfg["E"], cfg["CAP"], cfg["CBLK"]
    n_cores = cfg["n_cores"]
    F = 4 * D
    KD, FCH = D // P, F // P
    NCH = N // P
    B2 = 2 * NCH
    DHW = min(512, D)
    NDH = D // DHW
    bf16 = mybir.dt.np(mybir.dt.bfloat16)

    xf = np.ascontiguousarray(np.asarray(x, np.float32).reshape(N, D))
    xpad = np.concatenate([xf, np.zeros((1, D), np.float32)], axis=0)
    xT = np.ascontiguousarray(xf.T)
    wg = np.ascontiguousarray(np.asarray(w_g, np.float32))
    cfc_all = np.asarray(c_fc, np.float32)
    cpj_all = np.asarray(c_proj, np.float32)
    lnw = np.ascontiguousarray(np.broadcast_to(np.asarray(ln_w, np.float32), (P, D)))
    lnb = np.ascontiguousarray(np.broadcast_to(np.asarray(ln_b, np.float32), (P, D)))

    in_maps = []
    for e in range(n_cores):
        cfc_t = np.ascontiguousarray(
            cfc_all[e].reshape(KD, P, FCH, P).transpose(2, 1, 0, 3))
        cpj_t = np.ascontiguousarray(
            cpj_all[e].reshape(FCH, P, NDH, DHW).transpose(2, 0, 1, 3)).astype(bf16)
        eb = np.full((E,), float(CAP), np.float32)
        eb[e] = 0.0
        ebias = np.ascontiguousarray(
            np.broadcast_to(np.tile(eb, B2), (P, B2 * E)))
        in_maps.append(dict(xpad=xpad, xT=xT, wg=wg, cfc=cfc_t, cpj=cpj_t,
                            ebias=ebias, lnw=lnw, lnb=lnb))
    return in_maps


_CACHE = {}


def _compiled_full():
    key = "full"
    if key not in _CACHE:
        _CACHE[key] = build_moe_kernel(**FULL_CFG)
    return _CACHE[key]


def run_on_hw(inputs, trace=False):
    """Runs the full-size kernel on the 8 NeuronCores. Returns (out, results)."""
    from concourse.bass_utils import run_bass_kernel_spmd

    cfg = FULL_CFG
    nc = _compiled_full()
    in_maps = prep_in_maps(inputs["x"], inputs["w_g"], inputs["c_fc"],
                           inputs["c_proj"], inputs["ln_w"], inputs["ln_b"], cfg)
    res = run_bass_kernel_spmd(nc, in_maps, list(range(cfg["n_cores"])),
                               trace=trace)
    shards = [res.results[i]["out"] for i in range(cfg["n_cores"])]
    out = np.concatenate(shards, axis=0).astype(np.float32)
    B, T = 4, 1024
    return out.reshape(B, T, cfg["D"]), res


def kernel(x, w_g, c_fc, c_proj, ln_w, ln_b):
    out, _ = run_on_hw(dict(x=x, w_g=w_g, c_fc=c_fc, c_proj=c_proj,
                            ln_w=ln_w, ln_b=ln_b))
    return out
